# revision 59
# baseline (speedup 1.0000x reference)
"""GQA attention kernel for 8 TRN2 NeuronCores (Bass/Tile, SPMD).

Sharding: core c -> (batch b = c // 4, kv-head kv = c % 4). Each core computes
the 4 query heads of its kv group for its batch and a partial (transposed)
output projection; the host sums the 4 partials per batch.

v3, on top of v2's fp16 software-pipelined emission (projection chains for
chunk j+1 and output-projection blocks for chunk j-1 interleaved into
attention phase j so the in-order PE queue never stalls on the ACT-paced
exp pipeline):
- every DRAM parameter is laid out so each DMA slice is one fully
  contiguous HBM block (the kernel head is HBM-bandwidth-bound);
- rope tables ship as their low 64 rows only and are duplicated on-chip;
- junk matmuls at kernel start and between the chunk-0 projection chains
  keep the PE's HAM clock gate at 2.4 GHz through the DMA lead-in;
- chunk-0's x quarters are split across both HWDGE rings (q2 slots
  between wk and wv on the sync ring) so each ring's delivery tracks the
  K-chain's consumption order;
- each head's softmax-denominator chain (ones-matmul -> reciprocal ->
  gpsimd broadcast -> scale) is deferred into the next head's stream so
  the in-order PE queue never waits on the DVE-accumulated P_sum;
- the last two output blocks ship as single-block stores so the final
  exec-gating DMA starts earlier.
Weights ride the sync HWDGE ring, x-chunks the scalar HWDGE ring,
constants the gpsimd SWDGE ring.
"""

import os
import sys

import numpy as np

for _p in ("/opt/trn_rl_repo", "/root/.axon_site/_ro/trn_rl_repo"):
    if os.path.isdir(_p) and _p not in sys.path:
        sys.path.insert(0, _p)

import concourse.bass as bass  # noqa: E402
import concourse.bass_isa as bass_isa  # noqa: E402
import concourse.mybir as mybir  # noqa: E402
from concourse import bacc  # noqa: E402
from concourse.tile import TileContext  # noqa: E402
from concourse.bass_utils import run_bass_kernel_spmd  # noqa: E402

B, T, D = 2, 2048, 2048
H, HKV, HD = 16, 4, 128
G = H // HKV            # query heads per kv head (= per core)
EQ = G * HD             # 512: query-projection rows per core
P = 128
TC = 512                # t-chunk (free dim of every matmul)
NJ = T // TC            # 4 chunks
DT = D // P             # 16 contraction tiles
DEPTH = 5               # exp/mask run this many S-tiles ahead of PV
SCALE = 1.0 / float(np.sqrt(HD))

F32 = mybir.dt.float32
F16 = mybir.dt.float16
EXP = mybir.ActivationFunctionType.Exp
RADD = bass_isa.ReduceOp.add

_CACHE = {}


def _build():
    nc = bacc.Bacc("TRN2", target_bir_lowering=False, debug=False)

    # All inputs arrive pre-transposed into SBUF layout (partition dim first,
    # contiguous per partition) so every DMA runs at full descriptor rate.
    # DRAM layouts put the SBUF partition dim innermost-but-one so every DMA
    # slice is one fully contiguous HBM block (strided reads at 4KB lines
    # were costing ~25% of effective DMA bandwidth in the DMA-bound head)
    xT = nc.declare_dram_parameter("xT", [NJ, 4, P, 4, TC], F16, isOutput=False)
    wqT = nc.declare_dram_parameter("wqT", [G, P, DT, HD], F16, isOutput=False)
    wkT = nc.declare_dram_parameter("wkT", [P, DT, HD], F16, isOutput=False)
    wvT = nc.declare_dram_parameter("wvT", [P, DT, HD], F16, isOutput=False)
    woT = nc.declare_dram_parameter("woT", [G, P, D], F16, isOutput=False)
    # rope tables are [freqs, freqs]-duplicated along hd: ship only the low
    # 64 rows and duplicate on-chip (halves their HBM footprint in the
    # DMA-bound head of the kernel)
    cosT = nc.declare_dram_parameter("cosT", [HD // 2, T], F16, isOutput=False)
    sinT = nc.declare_dram_parameter("sinT", [HD // 2, T], F16, isOutput=False)
    rmat = nc.declare_dram_parameter("rmat", [HD, HD], F16, isOutput=False)
    iden = nc.declare_dram_parameter("iden", [P, P], F16, isOutput=False)
    maskt = nc.declare_dram_parameter("maskt", [P, P], F16, isOutput=False)
    ones_k = nc.declare_dram_parameter("ones_k", [P, 1], F16, isOutput=False)
    # output stored as [chunk, block-pair, p, 2, t'] with y row =
    # (2*bp + i)*128 + p and t = chunk*TC + t' (host un-permutes): every
    # 0.25 MiB store is one fully contiguous HBM write
    yT = nc.declare_dram_parameter("yT", [NJ, DT // 2, P, 2, TC], F16,
                                   isOutput=True)

    with TileContext(nc) as tc:
        with (
            tc.tile_pool(name="const", bufs=1) as cst,
            tc.tile_pool(name="kv", bufs=1) as kvp,
            tc.tile_pool(name="ot", bufs=1) as otp,
            tc.tile_pool(name="wts", bufs=1) as wts,
            tc.tile_pool(name="xs", bufs=2) as xs,
            tc.tile_pool(name="qk", bufs=2) as qk,
            tc.tile_pool(name="vt", bufs=2) as vtp,
            tc.tile_pool(name="work", bufs=6) as wk,
            tc.tile_pool(name="rtmp", bufs=2) as rtmp,
            tc.tile_pool(name="ls", bufs=2) as lsp,
            tc.tile_pool(name="yout", bufs=6) as yop,
            tc.tile_pool(name="ps_acc", bufs=2, space="PSUM") as ps_acc,
            tc.tile_pool(name="ps_s", bufs=2, space="PSUM") as ps_s,
            tc.tile_pool(name="ps_o", bufs=2, space="PSUM") as ps_o,
            tc.tile_pool(name="ps_y", bufs=2, space="PSUM") as ps_y,
        ):
            # PE warm-up: the HAM clock gate keeps the PE at 1.2 GHz until it
            # has seen ~3.4us of sustained activity, and the first ~9us are
            # DMA lead-in with an idle PE. Burn junk matmuls on a memset
            # scratch tile so the array is at 2.4 GHz when real work arrives.
            warm_sb = cst.tile([P, 64], F16, tag="warm")
            nc.gpsimd.memset(warm_sb[:], 0)
            warm_ps = ps_s.tile([64, 32], F32, tag="s", name="warm_ps")
            for _ in range(44):
                nc.tensor.matmul(warm_ps[:], warm_sb[:, :64], warm_sb[:, :32],
                                 start=True, stop=True)

            # Constants ride the gpsimd SWDGE ring so they don't delay the
            # weight/x loads on the two HWDGE rings.
            cos_sb = cst.tile([HD, T], F16, tag="cos")
            sin_sb = cst.tile([HD, T], F16, tag="sin")
            rmat_sb = cst.tile([HD, HD], F16, tag="rmat")
            iden_sb = cst.tile([P, P], F16, tag="iden")
            mask_sb = cst.tile([P, P], F16, tag="mask")
            onek_sb = cst.tile([P, 1], F16, tag="onek")
            HH = HD // 2

            def load_rope(c):
                # low 64 rows from HBM, then an SBUF->SBUF duplicate for the
                # high rows (cos/sin are [freqs, freqs] along hd)
                csl = slice(c * TC, (c + 1) * TC)
                nc.gpsimd.dma_start(cos_sb[:HH, csl], cosT[:, csl])
                nc.gpsimd.dma_start(sin_sb[:HH, csl], sinT[:, csl])
                nc.gpsimd.dma_start(cos_sb[HH:, csl], cos_sb[:HH, csl])
                nc.gpsimd.dma_start(sin_sb[HH:, csl], sin_sb[:HH, csl])

            # tiny constants + chunk-0 rope slices first; later chunks are
            # deferred into load_x so they don't compete with x/weights
            nc.gpsimd.dma_start(rmat_sb[:], rmat[:])
            nc.gpsimd.dma_start(iden_sb[:], iden[:])
            nc.gpsimd.dma_start(mask_sb[:], maskt[:])
            nc.gpsimd.dma_start(onek_sb[:], ones_k[:])
            load_rope(0)

            # Per-chunk K/V/attn-out tiles (separate tiles per chunk so the
            # interleaved emission never creates false whole-tile hazards
            # between phase B_j reads and phase A_{j+1} writes).
            kt_sbs = [kvp.tile([HD, TC], F16, tag=f"kt{c}", name=f"kt{c}")
                      for c in range(NJ)]
            v_sbs = [kvp.tile([P, 4, HD], F16, tag=f"v{c}", name=f"v{c}")
                     for c in range(NJ)]
            ot_js = [otp.tile([HD, G, TC], F16, tag=f"ot{c}", name=f"ot{c}")
                     for c in range(NJ)]

            wq_sb = wts.tile([P, G, DT, HD], F16, tag="wq")
            wk_sb = wts.tile([P, DT, HD], F16, tag="wk")
            wv_sb = wts.tile([P, DT, HD], F16, tag="wv")
            wo_sb = wts.tile([P, G, D], F16, tag="wo")

            def load_x(j):
                # x chunk j on the scalar HWDGE ring (weights keep the sync
                # ring to themselves). Quarter DMAs (4KB lines) issued
                # back-to-back in consumption order keep the ring deep.
                tiles = []
                for q in range(4):
                    xq = xs.tile([P, 4, TC], F16, tag=f"xc{q}", name=f"xc{q}")
                    nc.scalar.dma_start(xq[:], xT[j, q])
                    tiles.append(xq)
                if j > 0:
                    load_rope(j)
                return tiles


            def psum_s(shape=(P, TC), dtype=F32):
                return ps_s.tile(list(shape), dtype, tag="s", name="s")

            def finish_rope(s, t1, jsl):
                # s <- s*cos + rotate_half(s)*sin; t1 = s*cos precomputed
                pr = psum_s()
                nc.tensor.matmul(pr[:], rmat_sb[:], s, start=True, stop=True)
                nc.vector.tensor_mul(out=s, in0=pr[:], in1=sin_sb[:, jsl])
                nc.vector.tensor_add(out=s, in0=s, in1=t1[:])

            def a_thunks(j, xcq, pad=0):
                """Emission thunks for phase A_j, in order
                [K, V, Q0, transposes, Q1, Q2, Q3, flush]: the first four
                are everything attention head 0 of chunk j needs, so B_j
                can start while Q1..Q3 weave in as fillers. Each chain
                finishes the previous chain's rope (hides the psum
                eviction). `pad` junk matmuls after each chain keep the HAM
                clock gate warm through the just-in-time DMA arrivals of
                the first chunk's weights and x quarters."""
                jsl = slice(j * TC, (j + 1) * TC)
                qt = qk.tile([HD, G, TC], F16, tag="qt")
                vt = vtp.tile([HD, TC], F16, tag="vt")
                rope_q = []

                def chain(kind):
                    def emit():
                        acc = ps_acc.tile([P, TC], F32, tag="acc", name="acc")
                        for dt in range(DT):
                            if kind == "k":
                                lhsT = wk_sb[:, dt]
                            elif kind == "v":
                                lhsT = wv_sb[:, dt]
                            else:
                                lhsT = wq_sb[:, kind, dt]
                            nc.tensor.matmul(acc[:], lhsT,
                                             xcq[dt // 4][:, dt % 4],
                                             start=(dt == 0),
                                             stop=(dt == DT - 1))
                        if kind == "v":
                            nc.scalar.copy(vt[:], acc[:])
                        else:
                            s = kt_sbs[j][:] if kind == "k" else qt[:, kind]
                            nc.scalar.copy(s, acc[:])
                            t1 = rtmp.tile([HD, TC], F16, tag="t1")
                            nc.vector.tensor_mul(out=t1[:], in0=s,
                                                 in1=cos_sb[:, jsl])
                            rope_q.append((s, t1))
                        while len(rope_q) >= (1 if kind == "v" else 2):
                            finish_rope(*rope_q.pop(0), jsl)
                        if pad:
                            jp = psum_s()
                            for _ in range(pad):
                                nc.tensor.matmul(jp[:64, :32],
                                                 warm_sb[:, :64],
                                                 warm_sb[:, :32],
                                                 start=True, stop=True)
                    return emit

                def transposes():
                    while rope_q:
                        finish_rope(*rope_q.pop(0), jsl)
                    for tt in range(4):
                        pvt = psum_s((P, P), F16)
                        nc.tensor.transpose(pvt[:], vt[:, tt * P:(tt + 1) * P],
                                            iden_sb[:])
                        nc.vector.tensor_copy(v_sbs[j][:, tt], pvt[:])

                def flush():
                    while rope_q:
                        finish_rope(*rope_q.pop(0), jsl)

                thunks = [chain("k"), chain("v"), chain(0), transposes,
                          chain(1), chain(2), chain(3), flush]
                return thunks, qt

            def c_thunks(j):
                """Emission thunks for phase C_j: output projection of
                attention chunk j, one thunk per 128-row output block."""
                jsl = slice(j * TC, (j + 1) * TC)

                last = j == NJ - 1
                pair = {}

                def block(dt):
                    def emit():
                        # in the tail (no attention running) rotate across
                        # all three idle psum pools for a 6-deep pipeline
                        pool, tg = ([(ps_y, "y"), (ps_s, "s"),
                                     (ps_acc, "acc")][dt % 3]
                                    if last else (ps_y, "y"))
                        py = pool.tile([P, TC], F32, tag=tg, name="py")
                        for g in range(G):
                            nc.tensor.matmul(py[:],
                                             wo_sb[:, g, dt * P:(dt + 1) * P],
                                             ot_js[j][:, g],
                                             start=(g == 0), stop=(g == G - 1))
                        if dt % 2 == 0:
                            pair["t"] = yop.tile([P, 2, TC], F16, tag="ysb",
                                                 name="ysb")
                        y_sb = pair["t"]
                        # eviction engine per phase: C_j runs inside B_{j+1},
                        # whose ACT load grows with j — shift evictions from
                        # ACT (j=0) to DVE (j=2) accordingly
                        if j == 0:
                            ev = "act"
                        elif j == 2:
                            ev = "dve"
                        else:
                            ev = "act" if dt % 2 else "dve"
                        if ev == "dve":
                            nc.vector.tensor_copy(y_sb[:, dt % 2], py[:])
                        else:
                            nc.scalar.copy(y_sb[:, dt % 2], py[:])
                        if last and dt >= DT - 2:
                            # tail: ship the final blocks singly so the
                            # last (exec-gating) store starts earlier
                            nc.sync.dma_start(yT[j, dt // 2][:, dt % 2],
                                              y_sb[:, dt % 2])
                        elif dt % 2 == 1:
                            # two 128-row blocks per DMA: one fully
                            # contiguous 0.25 MiB HBM write
                            nc.sync.dma_start(yT[j, dt // 2], y_sb[:])
                    return emit

                return [block(dt) for dt in range(DT)]

            def emit_b(j, qt, fillers, need=None):
                """Attention for q-block j (all 4 heads), with `fillers`
                (independent emission thunks) woven in so the PE queue keeps
                streaming while exp paces the softmax pipeline. `need[h]`
                forces a minimum filler count before head h (for chunk-0's
                Q1..Q3 chains, which later heads depend on)."""
                jsl = slice(j * TC, (j + 1) * TC)
                nk = 4 * (j + 1)
                nfill = len(fillers)
                slots = G * nk
                fi = 0
                done = 0

                denoms = []

                def denom(po, psum16, h):
                    # softmax denominator: ones-matmul over P_sum, then
                    # reciprocal + partition broadcast + scale. Deferred
                    # into the NEXT head's stream so the (in-order) PE
                    # queue never waits on the DVE-accumulated P_sum.
                    # (A gpsimd partition_all_reduce instead measured
                    # ~2.5us each and stalled the whole pipeline.)
                    def emit():
                        pl = ps_acc.tile([1, TC], F32, tag="acc", name="pl")
                        nc.tensor.matmul(pl[:], onek_sb[:], psum16[:],
                                         start=True, stop=True)
                        rinv = lsp.tile([1, TC], F32, tag="rinv")
                        nc.vector.reciprocal_approx_fast(rinv[:], pl[:])
                        binv = lsp.tile([P, TC], F32, tag="binv")
                        nc.gpsimd.partition_broadcast(binv[:], rinv[:])
                        nc.vector.tensor_mul(out=ot_js[j][:, h], in0=po[:],
                                             in1=binv[:])
                    return emit

                for h in range(G):
                    while need is not None and fi < need[h]:
                        fillers[fi]()
                        fi += 1
                    po = ps_o.tile([P, TC], F32, tag="o", name="po")
                    psum16 = lsp.tile([P, TC], F16, tag="psum")
                    pipe = []

                    def drain():
                        ppt, pkt, pqs = pipe.pop(0)
                        nc.tensor.matmul(po[:, pqs], v_sbs[pkt // 4][:, pkt % 4],
                                         ppt[:, pqs],
                                         start=(pkt == 0), stop=(pkt == nk - 1))

                    for kt in range(nk):
                        m = kt - 4 * j
                        off = 0 if m < 0 else P * m
                        qs = slice(off, TC)
                        pss = psum_s()
                        c, q = kt // 4, kt % 4
                        nc.tensor.matmul(pss[:, qs],
                                         kt_sbs[c][:, q * P:(q + 1) * P],
                                         qt[:, h, qs], start=True, stop=True)
                        pt = wk.tile([P, TC], F16, tag="pt", bufs=7)
                        nc.scalar.activation(pt[:, qs], pss[:, qs], EXP,
                                             scale=SCALE)
                        if m >= 0:
                            ssl = slice(off, off + P)
                            nc.vector.tensor_mul(out=pt[:, ssl], in0=pt[:, ssl],
                                                 in1=mask_sb[:])
                        if kt == 0:
                            nc.vector.tensor_copy(psum16[:], pt[:])
                        else:
                            nc.vector.tensor_add(out=psum16[:, qs],
                                                 in0=psum16[:, qs],
                                                 in1=pt[:, qs])
                        pipe.append((pt, kt, qs))
                        if len(pipe) > DEPTH:
                            drain()
                        if kt == 2 and denoms:
                            denoms.pop(0)()
                        done += 1
                        want = nfill * done // slots
                        while fi < want:
                            fillers[fi]()
                            fi += 1
                    while pipe:
                        drain()
                    denoms.append(denom(po, psum16, h))
                while denoms:
                    denoms.pop(0)()
                while fi < nfill:
                    fillers[fi]()
                    fi += 1

            # ---- emission schedule -------------------------------------
            # DMA issues go out big and in consumption order, with chunk-0's
            # x quarters split across BOTH HWDGE rings so each ring's
            # delivery tracks the K-chain's consumption rate: scalar carries
            # q0/q1/q3, sync slots q2 between wk and wv. Deep rings saturate
            # HBM; the PE warm-up absorbs the first-arrival latency.
            xcq = [xs.tile([P, 4, TC], F16, tag=f"xc{q}", name=f"xc{q}")
                   for q in range(4)]
            nc.scalar.dma_start(xcq[0][:], xT[0, 0])
            nc.scalar.dma_start(xcq[1][:], xT[0, 1])
            nc.scalar.dma_start(xcq[3][:], xT[0, 3])
            nc.sync.dma_start(wk_sb[:], wkT[:])
            nc.sync.dma_start(xcq[2][:], xT[0, 2])
            nc.sync.dma_start(wv_sb[:], wvT[:])
            for h in range(G):
                nc.sync.dma_start(wq_sb[:, h], wqT[h])

            athk0, qt = a_thunks(0, xcq, pad=6)
            for t in athk0[:4]:
                t()
            for g in range(G):
                nc.sync.dma_start(wo_sb[:, g], woT[g])
            carry = []
            for j in range(NJ):
                fillers = (list(carry) if j > 0 else list(athk0[4:]))
                carry = []
                need = None if j > 0 else [0, 2, 3, 4]
                if j + 1 < NJ:
                    xcq = load_x(j + 1)
                    athk, qt_next = a_thunks(j + 1, xcq)
                    fillers += athk
                else:
                    qt_next = None
                if j > 0:
                    fillers += c_thunks(j - 1)
                emit_b(j, qt, fillers, need=need)
                qt = qt_next
            for t in c_thunks(NJ - 1):
                t()

    nc.compile()
    return nc


def _host_shards(inputs):
    x = np.asarray(inputs["x"], dtype=np.float32)
    cos = np.asarray(inputs["cos"], dtype=np.float32)
    sin = np.asarray(inputs["sin"], dtype=np.float32)
    Wq = np.asarray(inputs["Wq"], dtype=np.float32)
    Wk = np.asarray(inputs["Wk"], dtype=np.float32)
    Wv = np.asarray(inputs["Wv"], dtype=np.float32)
    Wo = np.asarray(inputs["Wo"], dtype=np.float32)

    f16 = np.float16
    # rope tables are [freqs, freqs]-duplicated along hd: ship low rows only
    cosT = np.ascontiguousarray(cos.T[:HD // 2]).astype(f16)
    sinT = np.ascontiguousarray(sin.T[:HD // 2]).astype(f16)
    iden = np.eye(P, dtype=f16)
    # one lower-triangle mask block reused for every diagonal k-tile
    maskt = (np.arange(P)[None, :] >= np.arange(P)[:, None]).astype(f16)
    ones_k = np.ones((P, 1), f16)
    rmat = np.zeros((HD, HD), f16)
    half = HD // 2
    for i in range(half):
        rmat[i + half, i] = -1.0     # out[m<64] = -q[m+64]
        rmat[i, i + half] = 1.0      # out[m>=64] = q[m-64]

    def to_sbuf_layout(wT, cols):
        # [D_contract, cols] -> [P, D_contract//P, cols], partition dim first
        return np.ascontiguousarray(
            wT.reshape(-1, P, cols).transpose(1, 0, 2)).astype(f16)

    # x[b].T is [d, t]; device layout [j, q, p, dtq, t'] with d = (4q+dtq)*P+p
    # and t = j*TC + t' makes each (j, q) quarter-load one contiguous block.
    xTs = [np.ascontiguousarray(
        x[b].T.reshape(4, 4, P, NJ, TC).transpose(3, 0, 2, 1, 4)).astype(f16)
        for b in range(B)]
    wqTs = []
    for kv in range(HKV):
        per_h = [to_sbuf_layout(
            Wq[kv * EQ + h * HD: kv * EQ + (h + 1) * HD].T, HD)
            for h in range(G)]
        wqTs.append(np.ascontiguousarray(np.stack(per_h, axis=0)))
    wkTs = [to_sbuf_layout(Wk[kv * HD:(kv + 1) * HD].T, HD) for kv in range(HKV)]
    wvTs = [to_sbuf_layout(Wv[kv * HD:(kv + 1) * HD].T, HD) for kv in range(HKV)]
    woTs = [np.ascontiguousarray(
        to_sbuf_layout(Wo[:, kv * EQ:(kv + 1) * EQ].T, D).transpose(1, 0, 2))
        for kv in range(HKV)]

    in_maps = []
    for c in range(8):
        b, kv = divmod(c, HKV)
        in_maps.append({
            "xT": xTs[b], "wqT": wqTs[kv], "wkT": wkTs[kv], "wvT": wvTs[kv],
            "woT": woTs[kv], "cosT": cosT, "sinT": sinT, "rmat": rmat,
            "iden": iden, "maskt": maskt, "ones_k": ones_k,
        })
    return in_maps


def get_nc():
    if "nc" not in _CACHE:
        _CACHE["nc"] = _build()
    return _CACHE["nc"]


def run(inputs, **kw):
    nc = get_nc()
    in_maps = _host_shards(inputs)
    res = run_bass_kernel_spmd(nc, in_maps, core_ids=list(range(8)), **kw)
    out = np.zeros((B, T, D), np.float32)
    for c in range(8):
        b = c // HKV
        # yT is [j, bp, p, i, t'] with y row = (2*bp + i)*128 + p and
        # t = j*TC + t'
        y = res.results[c]["yT"].astype(np.float32) \
            .transpose(1, 3, 2, 0, 4).reshape(D, T)
        out[b] += y.T
    return out, res


def kernel(**inputs) -> np.ndarray:
    out, _ = run(inputs)
    return out



# revision 60
# speedup vs baseline: 1.0021x; 1.0021x over previous
"""GQA attention kernel for 8 TRN2 NeuronCores (Bass/Tile, SPMD).

Sharding: core c -> (batch b = c // 4, kv-head kv = c % 4). Each core computes
the 4 query heads of its kv group for its batch and a partial (transposed)
output projection; the host sums the 4 partials per batch.

v3, on top of v2's fp16 software-pipelined emission (projection chains for
chunk j+1 and output-projection blocks for chunk j-1 interleaved into
attention phase j so the in-order PE queue never stalls on the ACT-paced
exp pipeline):
- every DRAM parameter is laid out so each DMA slice is one fully
  contiguous HBM block (the kernel head is HBM-bandwidth-bound);
- rope tables ship as their low 64 rows only and are duplicated on-chip;
- junk matmuls at kernel start and between the chunk-0 projection chains
  keep the PE's HAM clock gate at 2.4 GHz through the DMA lead-in;
- chunk-0's x quarters are split across both HWDGE rings (q2 slots
  between wk and wv on the sync ring) so each ring's delivery tracks the
  K-chain's consumption order;
- each head's softmax-denominator chain (ones-matmul -> reciprocal ->
  gpsimd broadcast -> scale) is deferred into the next head's stream so
  the in-order PE queue never waits on the DVE-accumulated P_sum;
- the last two output blocks ship as single-block stores so the final
  exec-gating DMA starts earlier.
Weights ride the sync HWDGE ring, x-chunks the scalar HWDGE ring,
constants the gpsimd SWDGE ring.
"""

import os
import sys

import numpy as np

for _p in ("/opt/trn_rl_repo", "/root/.axon_site/_ro/trn_rl_repo"):
    if os.path.isdir(_p) and _p not in sys.path:
        sys.path.insert(0, _p)

import concourse.bass as bass  # noqa: E402
import concourse.bass_isa as bass_isa  # noqa: E402
import concourse.mybir as mybir  # noqa: E402
from concourse import bacc  # noqa: E402
from concourse.tile import TileContext  # noqa: E402
from concourse.bass_utils import run_bass_kernel_spmd  # noqa: E402

B, T, D = 2, 2048, 2048
H, HKV, HD = 16, 4, 128
G = H // HKV            # query heads per kv head (= per core)
EQ = G * HD             # 512: query-projection rows per core
P = 128
TC = 512                # t-chunk (free dim of every matmul)
NJ = T // TC            # 4 chunks
DT = D // P             # 16 contraction tiles
DEPTH = 4               # exp/mask run this many S-tiles ahead of PV
SCALE = 1.0 / float(np.sqrt(HD))

F32 = mybir.dt.float32
F16 = mybir.dt.float16
EXP = mybir.ActivationFunctionType.Exp
RADD = bass_isa.ReduceOp.add

_CACHE = {}


def _build():
    nc = bacc.Bacc("TRN2", target_bir_lowering=False, debug=False)

    # All inputs arrive pre-transposed into SBUF layout (partition dim first,
    # contiguous per partition) so every DMA runs at full descriptor rate.
    # DRAM layouts put the SBUF partition dim innermost-but-one so every DMA
    # slice is one fully contiguous HBM block (strided reads at 4KB lines
    # were costing ~25% of effective DMA bandwidth in the DMA-bound head)
    xT = nc.declare_dram_parameter("xT", [NJ, 4, P, 4, TC], F16, isOutput=False)
    wqT = nc.declare_dram_parameter("wqT", [G, P, DT, HD], F16, isOutput=False)
    wkT = nc.declare_dram_parameter("wkT", [P, DT, HD], F16, isOutput=False)
    wvT = nc.declare_dram_parameter("wvT", [P, DT, HD], F16, isOutput=False)
    woT = nc.declare_dram_parameter("woT", [G, P, D], F16, isOutput=False)
    # rope tables are [freqs, freqs]-duplicated along hd: ship only the low
    # 64 rows and duplicate on-chip (halves their HBM footprint in the
    # DMA-bound head of the kernel)
    cosT = nc.declare_dram_parameter("cosT", [HD // 2, T], F16, isOutput=False)
    sinT = nc.declare_dram_parameter("sinT", [HD // 2, T], F16, isOutput=False)
    rmat = nc.declare_dram_parameter("rmat", [HD, HD], F16, isOutput=False)
    iden = nc.declare_dram_parameter("iden", [P, P], F16, isOutput=False)
    maskt = nc.declare_dram_parameter("maskt", [P, P], F16, isOutput=False)
    ones_k = nc.declare_dram_parameter("ones_k", [P, 1], F16, isOutput=False)
    # output stored as [chunk, block-pair, p, 2, t'] with y row =
    # (2*bp + i)*128 + p and t = chunk*TC + t' (host un-permutes): every
    # 0.25 MiB store is one fully contiguous HBM write
    yT = nc.declare_dram_parameter("yT", [NJ, DT // 2, P, 2, TC], F16,
                                   isOutput=True)

    with TileContext(nc) as tc:
        with (
            tc.tile_pool(name="const", bufs=1) as cst,
            tc.tile_pool(name="kv", bufs=1) as kvp,
            tc.tile_pool(name="ot", bufs=1) as otp,
            tc.tile_pool(name="wts", bufs=1) as wts,
            tc.tile_pool(name="xs", bufs=2) as xs,
            tc.tile_pool(name="qk", bufs=2) as qk,
            tc.tile_pool(name="vt", bufs=2) as vtp,
            tc.tile_pool(name="work", bufs=6) as wk,
            tc.tile_pool(name="rtmp", bufs=2) as rtmp,
            tc.tile_pool(name="ls", bufs=2) as lsp,
            tc.tile_pool(name="yout", bufs=6) as yop,
            tc.tile_pool(name="ps_acc", bufs=2, space="PSUM") as ps_acc,
            tc.tile_pool(name="ps_s", bufs=2, space="PSUM") as ps_s,
            tc.tile_pool(name="ps_o", bufs=2, space="PSUM") as ps_o,
            tc.tile_pool(name="ps_y", bufs=2, space="PSUM") as ps_y,
        ):
            # PE warm-up: the HAM clock gate keeps the PE at 1.2 GHz until it
            # has seen ~3.4us of sustained activity, and the first ~9us are
            # DMA lead-in with an idle PE. Burn junk matmuls on a memset
            # scratch tile so the array is at 2.4 GHz when real work arrives.
            warm_sb = cst.tile([P, 64], F16, tag="warm")
            nc.gpsimd.memset(warm_sb[:], 0)
            warm_ps = ps_s.tile([64, 32], F32, tag="s", name="warm_ps")
            for _ in range(44):
                nc.tensor.matmul(warm_ps[:], warm_sb[:, :64], warm_sb[:, :32],
                                 start=True, stop=True)

            # Constants ride the gpsimd SWDGE ring so they don't delay the
            # weight/x loads on the two HWDGE rings.
            cos_sb = cst.tile([HD, T], F16, tag="cos")
            sin_sb = cst.tile([HD, T], F16, tag="sin")
            rmat_sb = cst.tile([HD, HD], F16, tag="rmat")
            iden_sb = cst.tile([P, P], F16, tag="iden")
            mask_sb = cst.tile([P, P], F16, tag="mask")
            onek_sb = cst.tile([P, 1], F16, tag="onek")
            HH = HD // 2

            def load_rope(c):
                # low 64 rows from HBM, then an SBUF->SBUF duplicate for the
                # high rows (cos/sin are [freqs, freqs] along hd)
                csl = slice(c * TC, (c + 1) * TC)
                nc.gpsimd.dma_start(cos_sb[:HH, csl], cosT[:, csl])
                nc.gpsimd.dma_start(sin_sb[:HH, csl], sinT[:, csl])
                nc.gpsimd.dma_start(cos_sb[HH:, csl], cos_sb[:HH, csl])
                nc.gpsimd.dma_start(sin_sb[HH:, csl], sin_sb[:HH, csl])

            # tiny constants + chunk-0 rope slices first; later chunks are
            # deferred into load_x so they don't compete with x/weights
            nc.gpsimd.dma_start(rmat_sb[:], rmat[:])
            nc.gpsimd.dma_start(iden_sb[:], iden[:])
            nc.gpsimd.dma_start(mask_sb[:], maskt[:])
            nc.gpsimd.dma_start(onek_sb[:], ones_k[:])
            load_rope(0)

            # Per-chunk K/V/attn-out tiles (separate tiles per chunk so the
            # interleaved emission never creates false whole-tile hazards
            # between phase B_j reads and phase A_{j+1} writes).
            kt_sbs = [kvp.tile([HD, TC], F16, tag=f"kt{c}", name=f"kt{c}")
                      for c in range(NJ)]
            v_sbs = [kvp.tile([P, 4, HD], F16, tag=f"v{c}", name=f"v{c}")
                     for c in range(NJ)]
            ot_js = [otp.tile([HD, G, TC], F16, tag=f"ot{c}", name=f"ot{c}")
                     for c in range(NJ)]

            wq_sb = wts.tile([P, G, DT, HD], F16, tag="wq")
            wk_sb = wts.tile([P, DT, HD], F16, tag="wk")
            wv_sb = wts.tile([P, DT, HD], F16, tag="wv")
            wo_sb = wts.tile([P, G, D], F16, tag="wo")

            def load_x(j):
                # x chunk j on the scalar HWDGE ring (weights keep the sync
                # ring to themselves). Quarter DMAs (4KB lines) issued
                # back-to-back in consumption order keep the ring deep.
                tiles = []
                for q in range(4):
                    xq = xs.tile([P, 4, TC], F16, tag=f"xc{q}", name=f"xc{q}")
                    nc.scalar.dma_start(xq[:], xT[j, q])
                    tiles.append(xq)
                if j > 0:
                    load_rope(j)
                return tiles


            def psum_s(shape=(P, TC), dtype=F32):
                return ps_s.tile(list(shape), dtype, tag="s", name="s")

            def finish_rope(s, t1, jsl):
                # s <- s*cos + rotate_half(s)*sin; t1 = s*cos precomputed
                pr = psum_s()
                nc.tensor.matmul(pr[:], rmat_sb[:], s, start=True, stop=True)
                nc.vector.tensor_mul(out=s, in0=pr[:], in1=sin_sb[:, jsl])
                nc.vector.tensor_add(out=s, in0=s, in1=t1[:])

            def a_thunks(j, xcq, pad=0):
                """Emission thunks for phase A_j, in order
                [K, V, Q0, transposes, Q1, Q2, Q3, flush]: the first four
                are everything attention head 0 of chunk j needs, so B_j
                can start while Q1..Q3 weave in as fillers. Each chain
                finishes the previous chain's rope (hides the psum
                eviction). `pad` junk matmuls after each chain keep the HAM
                clock gate warm through the just-in-time DMA arrivals of
                the first chunk's weights and x quarters."""
                jsl = slice(j * TC, (j + 1) * TC)
                qt = qk.tile([HD, G, TC], F16, tag="qt")
                vt = vtp.tile([HD, TC], F16, tag="vt")
                rope_q = []

                def chain(kind):
                    def emit():
                        acc = ps_acc.tile([P, TC], F32, tag="acc", name="acc")
                        for dt in range(DT):
                            if kind == "k":
                                lhsT = wk_sb[:, dt]
                            elif kind == "v":
                                lhsT = wv_sb[:, dt]
                            else:
                                lhsT = wq_sb[:, kind, dt]
                            nc.tensor.matmul(acc[:], lhsT,
                                             xcq[dt // 4][:, dt % 4],
                                             start=(dt == 0),
                                             stop=(dt == DT - 1))
                        if kind == "v":
                            nc.scalar.copy(vt[:], acc[:])
                        else:
                            s = kt_sbs[j][:] if kind == "k" else qt[:, kind]
                            nc.scalar.copy(s, acc[:])
                            t1 = rtmp.tile([HD, TC], F16, tag="t1")
                            nc.vector.tensor_mul(out=t1[:], in0=s,
                                                 in1=cos_sb[:, jsl])
                            rope_q.append((s, t1))
                        while len(rope_q) >= (1 if kind == "v" else 2):
                            finish_rope(*rope_q.pop(0), jsl)
                        if pad:
                            jp = psum_s()
                            for _ in range(pad):
                                nc.tensor.matmul(jp[:64, :32],
                                                 warm_sb[:, :64],
                                                 warm_sb[:, :32],
                                                 start=True, stop=True)
                    return emit

                def transposes():
                    while rope_q:
                        finish_rope(*rope_q.pop(0), jsl)
                    for tt in range(4):
                        pvt = psum_s((P, P), F16)
                        nc.tensor.transpose(pvt[:], vt[:, tt * P:(tt + 1) * P],
                                            iden_sb[:])
                        nc.vector.tensor_copy(v_sbs[j][:, tt], pvt[:])

                def flush():
                    while rope_q:
                        finish_rope(*rope_q.pop(0), jsl)

                thunks = [chain("k"), chain("v"), chain(0), transposes,
                          chain(1), chain(2), chain(3), flush]
                return thunks, qt

            def c_thunks(j):
                """Emission thunks for phase C_j: output projection of
                attention chunk j, one thunk per 128-row output block."""
                jsl = slice(j * TC, (j + 1) * TC)

                last = j == NJ - 1
                pair = {}

                def block(dt):
                    def emit():
                        # in the tail (no attention running) rotate across
                        # all three idle psum pools for a 6-deep pipeline
                        pool, tg = ([(ps_y, "y"), (ps_s, "s"),
                                     (ps_acc, "acc")][dt % 3]
                                    if last else (ps_y, "y"))
                        py = pool.tile([P, TC], F32, tag=tg, name="py")
                        for g in range(G):
                            nc.tensor.matmul(py[:],
                                             wo_sb[:, g, dt * P:(dt + 1) * P],
                                             ot_js[j][:, g],
                                             start=(g == 0), stop=(g == G - 1))
                        if dt % 2 == 0:
                            pair["t"] = yop.tile([P, 2, TC], F16, tag="ysb",
                                                 name="ysb")
                        y_sb = pair["t"]
                        # eviction engine per phase: C_j runs inside B_{j+1},
                        # whose ACT load grows with j — shift evictions from
                        # ACT (j=0) to DVE (j=2) accordingly
                        if j == 0:
                            ev = "act"
                        elif j == 2:
                            ev = "dve"
                        else:
                            ev = "act" if dt % 2 else "dve"
                        if ev == "dve":
                            nc.vector.tensor_copy(y_sb[:, dt % 2], py[:])
                        else:
                            nc.scalar.copy(y_sb[:, dt % 2], py[:])
                        if last and dt >= DT - 2:
                            # tail: ship the final blocks singly so the
                            # last (exec-gating) store starts earlier
                            nc.sync.dma_start(yT[j, dt // 2][:, dt % 2],
                                              y_sb[:, dt % 2])
                        elif dt % 2 == 1:
                            # two 128-row blocks per DMA: one fully
                            # contiguous 0.25 MiB HBM write
                            nc.sync.dma_start(yT[j, dt // 2], y_sb[:])
                    return emit

                return [block(dt) for dt in range(DT)]

            def emit_b(j, qt, fillers, need=None):
                """Attention for q-block j (all 4 heads), with `fillers`
                (independent emission thunks) woven in so the PE queue keeps
                streaming while exp paces the softmax pipeline. `need[h]`
                forces a minimum filler count before head h (for chunk-0's
                Q1..Q3 chains, which later heads depend on)."""
                jsl = slice(j * TC, (j + 1) * TC)
                nk = 4 * (j + 1)
                nfill = len(fillers)
                slots = G * nk
                fi = 0
                done = 0

                denoms = []

                def denom(po, psum16, h):
                    # softmax denominator: ones-matmul over P_sum, then
                    # reciprocal + partition broadcast + scale. Deferred
                    # into the NEXT head's stream so the (in-order) PE
                    # queue never waits on the DVE-accumulated P_sum.
                    # (A gpsimd partition_all_reduce instead measured
                    # ~2.5us each and stalled the whole pipeline.)
                    def emit():
                        pl = ps_acc.tile([1, TC], F32, tag="acc", name="pl")
                        nc.tensor.matmul(pl[:], onek_sb[:], psum16[:],
                                         start=True, stop=True)
                        rinv = lsp.tile([1, TC], F32, tag="rinv")
                        nc.vector.reciprocal_approx_fast(rinv[:], pl[:])
                        binv = lsp.tile([P, TC], F32, tag="binv")
                        nc.gpsimd.partition_broadcast(binv[:], rinv[:])
                        nc.vector.tensor_mul(out=ot_js[j][:, h], in0=po[:],
                                             in1=binv[:])
                    return emit

                for h in range(G):
                    while need is not None and fi < need[h]:
                        fillers[fi]()
                        fi += 1
                    po = ps_o.tile([P, TC], F32, tag="o", name="po")
                    psum16 = lsp.tile([P, TC], F16, tag="psum")
                    pipe = []

                    def drain():
                        ppt, pkt, pqs = pipe.pop(0)
                        nc.tensor.matmul(po[:, pqs], v_sbs[pkt // 4][:, pkt % 4],
                                         ppt[:, pqs],
                                         start=(pkt == 0), stop=(pkt == nk - 1))

                    for kt in range(nk):
                        m = kt - 4 * j
                        off = 0 if m < 0 else P * m
                        qs = slice(off, TC)
                        pss = psum_s()
                        c, q = kt // 4, kt % 4
                        nc.tensor.matmul(pss[:, qs],
                                         kt_sbs[c][:, q * P:(q + 1) * P],
                                         qt[:, h, qs], start=True, stop=True)
                        pt = wk.tile([P, TC], F16, tag="pt")
                        nc.scalar.activation(pt[:, qs], pss[:, qs], EXP,
                                             scale=SCALE)
                        if m >= 0:
                            ssl = slice(off, off + P)
                            nc.vector.tensor_mul(out=pt[:, ssl], in0=pt[:, ssl],
                                                 in1=mask_sb[:])
                        if kt == 0:
                            nc.vector.tensor_copy(psum16[:], pt[:])
                        else:
                            nc.vector.tensor_add(out=psum16[:, qs],
                                                 in0=psum16[:, qs],
                                                 in1=pt[:, qs])
                        pipe.append((pt, kt, qs))
                        if len(pipe) > DEPTH:
                            drain()
                        if kt == 2 and denoms:
                            denoms.pop(0)()
                        done += 1
                        want = nfill * done // slots
                        while fi < want:
                            fillers[fi]()
                            fi += 1
                    while pipe:
                        drain()
                    denoms.append(denom(po, psum16, h))
                while denoms:
                    denoms.pop(0)()
                while fi < nfill:
                    fillers[fi]()
                    fi += 1

            # ---- emission schedule -------------------------------------
            # DMA issues go out big and in consumption order, with chunk-0's
            # x quarters split across BOTH HWDGE rings so each ring's
            # delivery tracks the K-chain's consumption rate: scalar carries
            # q0/q1/q3, sync slots q2 between wk and wv. Deep rings saturate
            # HBM; the PE warm-up absorbs the first-arrival latency.
            xcq = [xs.tile([P, 4, TC], F16, tag=f"xc{q}", name=f"xc{q}")
                   for q in range(4)]
            nc.scalar.dma_start(xcq[0][:], xT[0, 0])
            nc.scalar.dma_start(xcq[1][:], xT[0, 1])
            nc.scalar.dma_start(xcq[3][:], xT[0, 3])
            nc.sync.dma_start(wk_sb[:], wkT[:])
            nc.sync.dma_start(xcq[2][:], xT[0, 2])
            nc.sync.dma_start(wv_sb[:], wvT[:])
            for h in range(G):
                nc.sync.dma_start(wq_sb[:, h], wqT[h])

            athk0, qt = a_thunks(0, xcq, pad=6)
            for t in athk0[:4]:
                t()
            for g in range(G):
                nc.sync.dma_start(wo_sb[:, g], woT[g])
            carry = []
            for j in range(NJ):
                fillers = (list(carry) if j > 0 else list(athk0[4:]))
                carry = []
                need = None if j > 0 else [0, 2, 3, 4]
                if j + 1 < NJ:
                    xcq = load_x(j + 1)
                    athk, qt_next = a_thunks(j + 1, xcq)
                    fillers += athk
                else:
                    qt_next = None
                if j > 0:
                    fillers += c_thunks(j - 1)
                emit_b(j, qt, fillers, need=need)
                qt = qt_next
            for t in c_thunks(NJ - 1):
                t()

    nc.compile()
    return nc


def _host_shards(inputs):
    x = np.asarray(inputs["x"], dtype=np.float32)
    cos = np.asarray(inputs["cos"], dtype=np.float32)
    sin = np.asarray(inputs["sin"], dtype=np.float32)
    Wq = np.asarray(inputs["Wq"], dtype=np.float32)
    Wk = np.asarray(inputs["Wk"], dtype=np.float32)
    Wv = np.asarray(inputs["Wv"], dtype=np.float32)
    Wo = np.asarray(inputs["Wo"], dtype=np.float32)

    f16 = np.float16
    # rope tables are [freqs, freqs]-duplicated along hd: ship low rows only
    cosT = np.ascontiguousarray(cos.T[:HD // 2]).astype(f16)
    sinT = np.ascontiguousarray(sin.T[:HD // 2]).astype(f16)
    iden = np.eye(P, dtype=f16)
    # one lower-triangle mask block reused for every diagonal k-tile
    maskt = (np.arange(P)[None, :] >= np.arange(P)[:, None]).astype(f16)
    ones_k = np.ones((P, 1), f16)
    rmat = np.zeros((HD, HD), f16)
    half = HD // 2
    for i in range(half):
        rmat[i + half, i] = -1.0     # out[m<64] = -q[m+64]
        rmat[i, i + half] = 1.0      # out[m>=64] = q[m-64]

    def to_sbuf_layout(wT, cols):
        # [D_contract, cols] -> [P, D_contract//P, cols], partition dim first
        return np.ascontiguousarray(
            wT.reshape(-1, P, cols).transpose(1, 0, 2)).astype(f16)

    # x[b].T is [d, t]; device layout [j, q, p, dtq, t'] with d = (4q+dtq)*P+p
    # and t = j*TC + t' makes each (j, q) quarter-load one contiguous block.
    xTs = [np.ascontiguousarray(
        x[b].T.reshape(4, 4, P, NJ, TC).transpose(3, 0, 2, 1, 4)).astype(f16)
        for b in range(B)]
    wqTs = []
    for kv in range(HKV):
        per_h = [to_sbuf_layout(
            Wq[kv * EQ + h * HD: kv * EQ + (h + 1) * HD].T, HD)
            for h in range(G)]
        wqTs.append(np.ascontiguousarray(np.stack(per_h, axis=0)))
    wkTs = [to_sbuf_layout(Wk[kv * HD:(kv + 1) * HD].T, HD) for kv in range(HKV)]
    wvTs = [to_sbuf_layout(Wv[kv * HD:(kv + 1) * HD].T, HD) for kv in range(HKV)]
    woTs = [np.ascontiguousarray(
        to_sbuf_layout(Wo[:, kv * EQ:(kv + 1) * EQ].T, D).transpose(1, 0, 2))
        for kv in range(HKV)]

    in_maps = []
    for c in range(8):
        b, kv = divmod(c, HKV)
        in_maps.append({
            "xT": xTs[b], "wqT": wqTs[kv], "wkT": wkTs[kv], "wvT": wvTs[kv],
            "woT": woTs[kv], "cosT": cosT, "sinT": sinT, "rmat": rmat,
            "iden": iden, "maskt": maskt, "ones_k": ones_k,
        })
    return in_maps


def get_nc():
    if "nc" not in _CACHE:
        _CACHE["nc"] = _build()
    return _CACHE["nc"]


def run(inputs, **kw):
    nc = get_nc()
    in_maps = _host_shards(inputs)
    res = run_bass_kernel_spmd(nc, in_maps, core_ids=list(range(8)), **kw)
    out = np.zeros((B, T, D), np.float32)
    for c in range(8):
        b = c // HKV
        # yT is [j, bp, p, i, t'] with y row = (2*bp + i)*128 + p and
        # t = j*TC + t'
        y = res.results[c]["yT"].astype(np.float32) \
            .transpose(1, 3, 2, 0, 4).reshape(D, T)
        out[b] += y.T
    return out, res


def kernel(**inputs) -> np.ndarray:
    out, _ = run(inputs)
    return out

